# revision 6
# baseline (speedup 1.0000x reference)
"""Graphormer attention head on 8 trn2 NeuronCores (row-parallel), v2.

out = softmax(mask(q@k.T/8, adj)) @ v with q/k/v = x@W+b, adj scattered
from edge_index.

Core c owns output rows [c*1024, (c+1)*1024). All-fp16 single-term score
matmuls (error budget allows it), row-tiled in pairs across PE row-groups
0-63/64-127 (K=64 contraction -> 2 concurrent matmuls). K^T/Q^T are
duplicated onto both partition halves for free via duplicated weight
columns. The adjacency mask is host-built {0,448} fp8; a third of tile
pairs apply it on the PE as an accumulating identity-matmul into the
score PSUM (exp bias -448 then kills non-edges), the rest multiply
post-exp on the DVE (scaled by 1/448). exp runs on ScalarE over a
6-bank rotating PSUM window, batched 2 tiles per call when the slots
are contiguous. Softmax denominator via a ones-column appended to V.
"""
import os
import sys

for _p in ("/opt/trn_rl_repo", "/root/.axon_site/_ro/trn_rl_repo"):
    if os.path.isdir(_p) and _p not in sys.path:
        sys.path.insert(0, _p)

import numpy as np
import ml_dtypes

import concourse.bass as bass
import concourse.bacc as bacc
import concourse.mybir as mybir
import concourse.tile as tile
from concourse.bass_utils import run_bass_kernel_spmd

N = 8192
DIN = 256
DQ = 64
NCORES = 8
NLOC = N // NCORES          # 1024 rows per core
JT = N // 128               # 64 column tiles of 128
F32 = mybir.dt.float32
F16 = mybir.dt.float16
FP8 = mybir.dt.float8e4
MBIG = 240.0                # TRN fp8_exp4 max normal (OCP-compatible range)

# Pairs (2p, 2p+1) with p % PE_MASK_MOD == 0 apply the mask on the PE
# (identity-matmul into PSUM); the rest multiply on the DVE after exp.
PE_MASK_MOD = 3


def _emit(nc, tc, ctx):
    from concourse.mybir import AluOpType as AO, ActivationFunctionType as AF

    xt = nc.dram_tensor("xt", [DIN, N], F16, kind="ExternalInput")
    xtq = nc.dram_tensor("xtq", [DIN, NLOC], F16, kind="ExternalInput")
    wqd = nc.dram_tensor("wqd", [DIN, 128], F16, kind="ExternalInput")
    wkd = nc.dram_tensor("wkd", [DIN, 128], F16, kind="ExternalInput")
    wv = nc.dram_tensor("wv", [DIN, DQ], F16, kind="ExternalInput")
    bqd = nc.dram_tensor("bqd", [128, 1], F32, kind="ExternalInput")
    bkd = nc.dram_tensor("bkd", [128, 1], F32, kind="ExternalInput")
    i65 = nc.dram_tensor("i65", [DQ + 1, DQ + 1], F16, kind="ExternalInput")
    i128 = nc.dram_tensor("i128", [128, 128], FP8, kind="ExternalInput")
    maskt = nc.dram_tensor("maskt", [N, NLOC], FP8, kind="ExternalInput")
    out = nc.dram_tensor("out", [NLOC, DQ], F32, kind="ExternalOutput")

    pers = ctx.enter_context(tc.tile_pool(name="pers", bufs=1))
    pm = ctx.enter_context(tc.tile_pool(name="pm", bufs=3))
    pe_ = ctx.enter_context(tc.tile_pool(name="pe", bufs=2))
    pw = ctx.enter_context(tc.tile_pool(name="pw", bufs=3))
    pfin = ctx.enter_context(tc.tile_pool(name="pfin", bufs=2))
    psB = ctx.enter_context(tc.tile_pool(name="psB", bufs=1, space="PSUM"))
    pacc = ctx.enter_context(tc.tile_pool(name="pacc", bufs=1, space="PSUM"))

    # ---- persistent SBUF ----
    xt_sb = [pers.tile([128, N], F16, tag=f"xt{c}", name=f"xt{c}") for c in range(2)]
    xtq_sb = [pers.tile([128, NLOC], F16, tag=f"xtq{c}", name=f"xtq{c}")
              for c in range(2)]
    wqd_sb = [pers.tile([128, 128], F16, tag=f"wqd{c}", name=f"wqd{c}")
              for c in range(2)]
    wkd_sb = [pers.tile([128, 128], F16, tag=f"wkd{c}", name=f"wkd{c}")
              for c in range(2)]
    wv_sb = [pers.tile([128, DQ], F16, tag=f"wv{c}", name=f"wv{c}")
             for c in range(2)]
    for c in range(2):
        nc.sync.dma_start(wqd_sb[c][:], wqd[c * 128:(c + 1) * 128, :])
        nc.sync.dma_start(wkd_sb[c][:], wkd[c * 128:(c + 1) * 128, :])
        nc.sync.dma_start(wv_sb[c][:], wv[c * 128:(c + 1) * 128, :])
    bqd_sb = pers.tile([128, 1], F32, tag="bqd")
    bkd_sb = pers.tile([128, 1], F32, tag="bkd")
    i65_sb = pers.tile([DQ + 1, DQ + 1], F16, tag="i65")
    i128_sb = pers.tile([128, 128], FP8, tag="i128")
    nc.sync.dma_start(bqd_sb[:], bqd[:])
    nc.sync.dma_start(bkd_sb[:], bkd[:])
    nc.sync.dma_start(i65_sb[:], i65[:])
    nc.sync.dma_start(i128_sb[:], i128[:])

    nbig_sb = pers.tile([128, 1], F32, tag="nbig")      # -MBIG exp bias
    nc.vector.memset(nbig_sb[:], -MBIG)
    kth_sb = pers.tile([128, N], F16, tag="kth")        # K^T duplicated halves
    qth_sb = pers.tile([128, NLOC], F16, tag="qth")     # Q^T duplicated halves
    vh_sb = pers.tile([128, JT * (DQ + 1)], F16, tag="vh")
    accT_sb = pers.tile([DQ + 1, NLOC], F16, tag="accT")

    # x^T streamed in 1024-col chunks so projections can start early
    for c in range(2):
        nc.sync.dma_start(xtq_sb[c][:], xtq[c * 128:(c + 1) * 128, :])
        for s in range(N // NLOC):
            nc.sync.dma_start(
                xt_sb[c][:, s * NLOC:(s + 1) * NLOC],
                xt[c * 128:(c + 1) * 128, s * NLOC:(s + 1) * NLOC],
            )

    # big rotating PSUM window: 3 slots x 1024 fp32 (6 banks)
    sbig = psB.tile([128, 3 * NLOC], F32, tag="sbig")
    acc = pacc.tile([DQ + 1, NLOC], F32, tag="acc")

    # ---- projections (all-fp16 moving operands) ----
    # Q^T [128, 1024]: rows 0-63 = Q^T, 64-127 = copy (wqd has wq twice)
    qp = sbig[:, 0:NLOC]
    for b in range(2):
        o = qp[:, b * 512:(b + 1) * 512]
        nc.tensor.matmul(o, wqd_sb[0][:], xtq_sb[0][:, b * 512:(b + 1) * 512],
                         start=True, stop=False)
        nc.tensor.matmul(o, wqd_sb[1][:], xtq_sb[1][:, b * 512:(b + 1) * 512],
                         start=False, stop=True)
    nc.vector.tensor_scalar_add(qth_sb[:], qp, bqd_sb[:])

    # K^T [128, 8192] in 8 segments, alternating psum slots 1/2
    for s in range(8):
        kp = sbig[:, (1 + s % 2) * NLOC:(2 + s % 2) * NLOC]
        for b in range(2):
            o = kp[:, b * 512:(b + 1) * 512]
            cols = slice(s * NLOC + b * 512, s * NLOC + (b + 1) * 512)
            nc.tensor.matmul(o, wkd_sb[0][:], xt_sb[0][:, cols],
                             start=True, stop=False)
            nc.tensor.matmul(o, wkd_sb[1][:], xt_sb[1][:, cols],
                             start=False, stop=True)
        nc.vector.tensor_scalar_add(kth_sb[:, s * NLOC:(s + 1) * NLOC], kp,
                                    bkd_sb[:])

    # V as 64 blocks of [128, 65] (65th col = 1.0 for the denominator);
    # stationary = x^T block, moving = wv. bv folded in via i65 at the end.
    vh3 = vh_sb[:].rearrange("p (b e) -> p b e", e=DQ + 1)
    nc.vector.memset(vh3[:, :, DQ:DQ + 1], 1.0)
    for g in range(8):
        vp = sbig[:, 2 * NLOC + (g % 2) * 512:2 * NLOC + (g % 2 + 1) * 512]
        for b in range(8):
            jt = g * 8 + b
            o = vp[:, b * DQ:(b + 1) * DQ]
            nc.tensor.matmul(o, xt_sb[0][:, jt * 128:(jt + 1) * 128],
                             wv_sb[0][:], start=True, stop=False)
            nc.tensor.matmul(o, xt_sb[1][:, jt * 128:(jt + 1) * 128],
                             wv_sb[1][:], start=False, stop=True)
        nc.vector.tensor_copy(vh3[:, g * 8:(g + 1) * 8, 0:DQ], vp)

    # ---- main loop over 32 tile pairs ----
    mt3 = maskt.rearrange("(j p) c -> j p c", p=128)
    for p in range(JT // 2):
        jta, jtb = 2 * p, 2 * p + 1
        sla, slb = jta % 3, jtb % 3
        sa = sbig[:, sla * NLOC:(sla + 1) * NLOC]
        sb = sbig[:, slb * NLOC:(slb + 1) * NLOC]
        pe_mask = (p % PE_MASK_MOD == 0)

        m2 = pm.tile([128, 2 * NLOC], FP8, tag="m")
        m2v = m2[:].rearrange("p (t c) -> p t c", t=2)
        nc.sync.dma_start(m2v[:, 0, :], mt3[jta])
        nc.sync.dma_start(m2v[:, 1, :], mt3[jtb])

        # scores: row-tiled pair (A on PE rows 0-63, B on rows 64-127)
        kh_a = kth_sb[0:64, jta * 128:(jta + 1) * 128]
        kh_b = kth_sb[64:128, jtb * 128:(jtb + 1) * 128]
        for b in range(2):
            hs = slice(b * 512, (b + 1) * 512)
            nc.tensor.matmul(sa[:, hs], kh_a, qth_sb[0:64, hs],
                             start=True, stop=not pe_mask)
            nc.tensor.matmul(sb[:, hs], kh_b, qth_sb[64:128, hs],
                             start=True, stop=not pe_mask)
        if pe_mask:
            for st, mo in ((sa, 0), (sb, NLOC)):
                for b in range(2):
                    hs = slice(b * 512, (b + 1) * 512)
                    nc.tensor.matmul(st[:, hs], i128_sb[:],
                                     m2[:, mo + b * 512:mo + (b + 1) * 512],
                                     start=False, stop=True)

        # exp on ScalarE, batched when the pair's slots are contiguous
        dst_pool = pw if pe_mask else pe_
        d2 = dst_pool.tile([128, 2 * NLOC], F16, tag="d")
        bias = nbig_sb[:] if pe_mask else 0.0
        if slb == sla + 1:
            nc.scalar.activation(d2[:], sbig[:, sla * NLOC:(sla + 2) * NLOC],
                                 AF.Exp, bias=bias)
        else:
            nc.scalar.activation(d2[:, 0:NLOC], sa, AF.Exp, bias=bias)
            nc.scalar.activation(d2[:, NLOC:2 * NLOC], sb, AF.Exp, bias=bias)
        if pe_mask:
            w2 = d2
        else:
            w2 = pw.tile([128, 2 * NLOC], F16, tag="d")
            nc.vector.scalar_tensor_tensor(w2[:], d2[:], 1.0 / MBIG, m2[:],
                                           AO.mult, AO.mult)

        # attn @ [v | 1]
        for jt, wo in ((jta, 0), (jtb, NLOC)):
            vhb = vh3[:, jt, :]
            for b in range(2):
                nc.tensor.matmul(acc[:, b * 512:(b + 1) * 512], vhb,
                                 w2[:, wo + b * 512:wo + (b + 1) * 512],
                                 start=(jt == 0), stop=(jt == JT - 1))

    # ---- finish: transpose via matmul with I65 (adds bv*Z), divide by Z ----
    nc.vector.tensor_copy(accT_sb[:], acc[:])
    for it in range(NLOC // 128):
        po = sbig[:, it % 2 * NLOC:it % 2 * NLOC + DQ + 1]
        nc.tensor.matmul(po, accT_sb[:, it * 128:(it + 1) * 128], i65_sb[:],
                         start=True, stop=True)
        rz = pfin.tile([128, 1], F32, tag="rz")
        nc.vector.reciprocal(rz[:], po[:, DQ:DQ + 1])
        o_t = pfin.tile([128, DQ], F32, tag="o")
        nc.vector.tensor_scalar_mul(o_t[:], po[:, 0:DQ], rz[:])
        nc.sync.dma_start(out[it * 128:(it + 1) * 128, :], o_t[:])


_CACHE = {}


def _program():
    if "nc" not in _CACHE:
        import contextlib
        nc = bacc.Bacc("TRN2", target_bir_lowering=False, debug=False,
                       num_devices=NCORES)
        with tile.TileContext(nc) as tc:
            with contextlib.ExitStack() as ctx:
                _emit(nc, tc, ctx)
        nc.compile()
        _CACHE["nc"] = nc
    return _CACHE["nc"]


def kernel(**inputs):
    x = np.asarray(inputs["x"], dtype=np.float32)
    ei = np.asarray(inputs["edge_index"])
    Wq = np.asarray(inputs["Wq"], dtype=np.float32)
    bq = np.asarray(inputs["bq"], dtype=np.float32)
    Wk = np.asarray(inputs["Wk"], dtype=np.float32)
    bk = np.asarray(inputs["bk"], dtype=np.float32)
    Wv = np.asarray(inputs["Wv"], dtype=np.float32)
    bv = np.asarray(inputs["bv"], dtype=np.float32)

    scale = 1.0 / np.sqrt(np.float32(DQ))
    f16 = ml_dtypes.float16 if not hasattr(np, "float16") else np.float16
    xT = np.ascontiguousarray(x.T).astype(np.float16)        # (256, 8192)
    wq_s = (Wq * scale).astype(np.float16)
    wqd = np.ascontiguousarray(np.concatenate([wq_s, wq_s], axis=1))
    wk16 = Wk.astype(np.float16)
    wkd = np.ascontiguousarray(np.concatenate([wk16, wk16], axis=1))
    wv16 = np.ascontiguousarray(Wv.astype(np.float16))
    bqd = np.ascontiguousarray(np.tile(bq * scale, 2).reshape(128, 1))
    bkd = np.ascontiguousarray(np.tile(bk, 2).reshape(128, 1))
    i65 = np.eye(DQ + 1, dtype=np.float32)
    i65[DQ, :DQ] = bv
    i65 = i65.astype(np.float16)
    i128 = np.eye(128, dtype=np.float32).astype(ml_dtypes.float8_e4m3)
    adj = np.zeros((N, N), dtype=np.bool_)
    adj[ei[0], ei[1]] = True

    in_maps = []
    for c in range(NCORES):
        rows = slice(c * NLOC, (c + 1) * NLOC)
        in_maps.append({
            "xt": xT,
            "xtq": np.ascontiguousarray(xT[:, rows]),
            "wqd": wqd, "wkd": wkd, "wv": wv16,
            "bqd": bqd, "bkd": bkd, "i65": i65, "i128": i128,
            "maskt": np.ascontiguousarray(
                adj[rows].T.astype(np.float32) * MBIG).astype(
                ml_dtypes.float8_e4m3),
        })

    global _last_in_maps
    _last_in_maps = in_maps
    nc = _program()
    res = run_bass_kernel_spmd(nc, in_maps, core_ids=list(range(NCORES)))
    out = np.concatenate([res.results[c]["out"] for c in range(NCORES)], axis=0)
    return out.astype(np.float32)


_last_in_maps = None


# revision 11
# speedup vs baseline: 1.1457x; 1.1457x over previous
"""Graphormer attention head on 8 trn2 NeuronCores (row-parallel), v2.

out = softmax(mask(q@k.T/8, adj)) @ v with q/k/v = x@W+b, adj scattered
from edge_index.

Core c owns output rows [c*1024, (c+1)*1024). All-fp16 single-term score
matmuls (error budget allows it), row-tiled in pairs across PE row-groups
0-63/64-127 (K=64 contraction -> 2 concurrent matmuls). K^T/Q^T are
duplicated onto both partition halves for free via duplicated weight
columns. The adjacency mask is host-built {0,448} fp8; a third of tile
pairs apply it on the PE as an accumulating identity-matmul into the
score PSUM (exp bias -448 then kills non-edges), the rest multiply
post-exp on the DVE (scaled by 1/448). exp runs on ScalarE over a
6-bank rotating PSUM window, batched 2 tiles per call when the slots
are contiguous. Softmax denominator via a ones-column appended to V.
"""
import os
import sys

for _p in ("/opt/trn_rl_repo", "/root/.axon_site/_ro/trn_rl_repo"):
    if os.path.isdir(_p) and _p not in sys.path:
        sys.path.insert(0, _p)

import numpy as np
import ml_dtypes

import concourse.bass as bass
import concourse.bacc as bacc
import concourse.mybir as mybir
import concourse.tile as tile
from concourse.bass_utils import run_bass_kernel_spmd

N = 8192
DIN = 256
DQ = 64
NCORES = 8
NLOC = N // NCORES          # 1024 rows per core
JT = N // 128               # 64 column tiles of 128
F32 = mybir.dt.float32
F16 = mybir.dt.float16
FP8 = mybir.dt.float8e4
MBIG = 240.0                # TRN fp8_exp4 max normal (OCP-compatible range)

# Per-pair mask route: PE applies it as an identity-matmul into PSUM;
# DVE / GpSimd multiply after exp. Spread across the three engines.
def _mask_route(p):
    return ("PE", "DVE", "GP")[p % 3]


def _emit(nc, tc, ctx):
    from concourse.mybir import AluOpType as AO, ActivationFunctionType as AF

    xt = nc.dram_tensor("xt", [DIN, N], F16, kind="ExternalInput")
    xtq = nc.dram_tensor("xtq", [DIN, NLOC], F16, kind="ExternalInput")
    wqd = nc.dram_tensor("wqd", [DIN, 128], F16, kind="ExternalInput")
    wkd = nc.dram_tensor("wkd", [DIN, 128], F16, kind="ExternalInput")
    wv = nc.dram_tensor("wv", [DIN, DQ], F16, kind="ExternalInput")
    bqd = nc.dram_tensor("bqd", [128, 1], F32, kind="ExternalInput")
    bkd = nc.dram_tensor("bkd", [128, 1], F32, kind="ExternalInput")
    i65 = nc.dram_tensor("i65", [DQ + 1, DQ + 1], F16, kind="ExternalInput")
    i128 = nc.dram_tensor("i128", [128, 128], FP8, kind="ExternalInput")
    maskt = nc.dram_tensor("maskt", [N, NLOC], FP8, kind="ExternalInput")
    out = nc.dram_tensor("out", [NLOC, DQ], F32, kind="ExternalOutput")

    pers = ctx.enter_context(tc.tile_pool(name="pers", bufs=1))
    pm = ctx.enter_context(tc.tile_pool(name="pm", bufs=6))
    pe_ = ctx.enter_context(tc.tile_pool(name="pe", bufs=3))
    pw = ctx.enter_context(tc.tile_pool(name="pw", bufs=4))
    pfin = ctx.enter_context(tc.tile_pool(name="pfin", bufs=2))
    psB = ctx.enter_context(tc.tile_pool(name="psB", bufs=1, space="PSUM"))
    pacc = ctx.enter_context(tc.tile_pool(name="pacc", bufs=1, space="PSUM"))

    # ---- persistent SBUF ----
    xt_sb = [pers.tile([128, N], F16, tag=f"xt{c}", name=f"xt{c}") for c in range(2)]
    xtq_sb = [pers.tile([128, NLOC], F16, tag=f"xtq{c}", name=f"xtq{c}")
              for c in range(2)]
    wqd_sb = [pers.tile([128, 128], F16, tag=f"wqd{c}", name=f"wqd{c}")
              for c in range(2)]
    wkd_sb = [pers.tile([128, 128], F16, tag=f"wkd{c}", name=f"wkd{c}")
              for c in range(2)]
    wv_sb = [pers.tile([128, DQ], F16, tag=f"wv{c}", name=f"wv{c}")
             for c in range(2)]
    for c in range(2):
        nc.sync.dma_start(wqd_sb[c][:], wqd[c * 128:(c + 1) * 128, :])
        nc.sync.dma_start(wkd_sb[c][:], wkd[c * 128:(c + 1) * 128, :])
        nc.sync.dma_start(wv_sb[c][:], wv[c * 128:(c + 1) * 128, :])
    bqd_sb = pers.tile([128, 1], F32, tag="bqd")
    bkd_sb = pers.tile([128, 1], F32, tag="bkd")
    i65_sb = pers.tile([DQ + 1, DQ + 1], F16, tag="i65")
    i128_sb = pers.tile([128, 128], FP8, tag="i128")
    nc.sync.dma_start(bqd_sb[:], bqd[:])
    nc.sync.dma_start(bkd_sb[:], bkd[:])
    nc.sync.dma_start(i65_sb[:], i65[:])
    nc.sync.dma_start(i128_sb[:], i128[:])

    nbig_sb = pers.tile([128, 1], F32, tag="nbig")      # -MBIG exp bias
    nc.vector.memset(nbig_sb[:], -MBIG)
    lnbig_sb = pers.tile([128, 1], F32, tag="lnbig")    # -ln(MBIG) exp bias
    nc.vector.memset(lnbig_sb[:], -float(np.log(MBIG)))
    kth_sb = pers.tile([128, N], F16, tag="kth")        # K^T duplicated halves
    qth_sb = pers.tile([128, NLOC], F16, tag="qth")     # Q^T duplicated halves
    vh_sb = pers.tile([128, JT * (DQ + 1)], F16, tag="vh")
    accT_sb = pers.tile([DQ + 1, NLOC], F16, tag="accT")

    # x^T streamed in 1024-col chunks so projections can start early
    for c in range(2):
        nc.sync.dma_start(xtq_sb[c][:], xtq[c * 128:(c + 1) * 128, :])
        for s in range(N // NLOC):
            nc.sync.dma_start(
                xt_sb[c][:, s * NLOC:(s + 1) * NLOC],
                xt[c * 128:(c + 1) * 128, s * NLOC:(s + 1) * NLOC],
            )

    # big rotating PSUM window: 3 slots x 1024 fp32 (6 banks)
    sbig = psB.tile([128, 3 * NLOC], F32, tag="sbig")
    acc = pacc.tile([DQ + 1, NLOC], F32, tag="acc")

    # ---- projections (all-fp16 moving operands) ----
    # Q^T [128, 1024]: rows 0-63 = Q^T, 64-127 = copy (wqd has wq twice)
    qp = sbig[:, 0:NLOC]
    for b in range(2):
        o = qp[:, b * 512:(b + 1) * 512]
        nc.tensor.matmul(o, wqd_sb[0][:], xtq_sb[0][:, b * 512:(b + 1) * 512],
                         start=True, stop=False)
        nc.tensor.matmul(o, wqd_sb[1][:], xtq_sb[1][:, b * 512:(b + 1) * 512],
                         start=False, stop=True)
    nc.vector.tensor_scalar_add(qth_sb[:], qp, bqd_sb[:])

    # K^T [128, 8192] in 8 segments, alternating psum slots 1/2
    for s in range(8):
        kp = sbig[:, (1 + s % 2) * NLOC:(2 + s % 2) * NLOC]
        for b in range(2):
            o = kp[:, b * 512:(b + 1) * 512]
            cols = slice(s * NLOC + b * 512, s * NLOC + (b + 1) * 512)
            nc.tensor.matmul(o, wkd_sb[0][:], xt_sb[0][:, cols],
                             start=True, stop=False)
            nc.tensor.matmul(o, wkd_sb[1][:], xt_sb[1][:, cols],
                             start=False, stop=True)
        nc.vector.tensor_scalar_add(kth_sb[:, s * NLOC:(s + 1) * NLOC], kp,
                                    bkd_sb[:])

    # V as 64 blocks of [128, 65] (65th col = 1.0 for the denominator);
    # stationary = x^T block, moving = wv. bv folded in via i65 at the end.
    vh3 = vh_sb[:].rearrange("p (b e) -> p b e", e=DQ + 1)
    nc.vector.memset(vh3[:, :, DQ:DQ + 1], 1.0)
    for g in range(8):
        vp = sbig[:, 2 * NLOC + (g % 2) * 512:2 * NLOC + (g % 2 + 1) * 512]
        for b in range(8):
            jt = g * 8 + b
            o = vp[:, b * DQ:(b + 1) * DQ]
            nc.tensor.matmul(o, xt_sb[0][:, jt * 128:(jt + 1) * 128],
                             wv_sb[0][:], start=True, stop=False)
            nc.tensor.matmul(o, xt_sb[1][:, jt * 128:(jt + 1) * 128],
                             wv_sb[1][:], start=False, stop=True)
        nc.vector.tensor_copy(vh3[:, g * 8:(g + 1) * 8, 0:DQ], vp)

    # ---- main loop over 32 tile pairs, wv software-pipelined by one ----
    mt3 = maskt.rearrange("(j p) c -> j p c", p=128)

    def emit_wv(w2, jta, jtb):
        for jt, wo in ((jta, 0), (jtb, NLOC)):
            vhb = vh3[:, jt, :]
            for b in range(2):
                nc.tensor.matmul(acc[:, b * 512:(b + 1) * 512], vhb,
                                 w2[:, wo + b * 512:wo + (b + 1) * 512],
                                 start=(jt == 0), stop=(jt == JT - 1))

    prev = None
    for p in range(JT // 2):
        jta, jtb = 2 * p, 2 * p + 1
        sla, slb = jta % 3, jtb % 3
        sa = sbig[:, sla * NLOC:(sla + 1) * NLOC]
        sb = sbig[:, slb * NLOC:(slb + 1) * NLOC]
        route = _mask_route(p)
        pe_mask = route == "PE"

        m2 = pm.tile([128, 2 * NLOC], FP8, tag="m")
        nc.sync.dma_start(m2[:, 0:NLOC], mt3[jta])
        nc.sync.dma_start(m2[:, NLOC:2 * NLOC], mt3[jtb])

        # scores: row-tiled pair (A on PE rows 0-63, B on rows 64-127)
        kh_a = kth_sb[0:64, jta * 128:(jta + 1) * 128]
        kh_b = kth_sb[64:128, jtb * 128:(jtb + 1) * 128]
        for b in range(2):
            hs = slice(b * 512, (b + 1) * 512)
            nc.tensor.matmul(sa[:, hs], kh_a, qth_sb[0:64, hs],
                             start=True, stop=not pe_mask)
            nc.tensor.matmul(sb[:, hs], kh_b, qth_sb[64:128, hs],
                             start=True, stop=not pe_mask)
        if pe_mask:
            for st, mo in ((sa, 0), (sb, NLOC)):
                for b in range(2):
                    hs = slice(b * 512, (b + 1) * 512)
                    nc.tensor.matmul(st[:, hs], i128_sb[:],
                                     m2[:, mo + b * 512:mo + (b + 1) * 512],
                                     start=False, stop=True)

        # exp on ScalarE, batched when the pair's slots are contiguous
        dst_pool = pw if pe_mask else pe_
        d2 = dst_pool.tile([128, 2 * NLOC], F16, tag="d")
        if pe_mask:
            bias = nbig_sb[:]
        elif route == "GP":
            bias = lnbig_sb[:]      # exp(S)/MBIG; mask values MBIG restore it
        else:
            bias = 0.0
        if slb == sla + 1:
            nc.scalar.activation(d2[:], sbig[:, sla * NLOC:(sla + 2) * NLOC],
                                 AF.Exp, bias=bias)
        else:
            nc.scalar.activation(d2[:, 0:NLOC], sa, AF.Exp, bias=bias)
            nc.scalar.activation(d2[:, NLOC:2 * NLOC], sb, AF.Exp, bias=bias)
        if pe_mask:
            w2 = d2
        else:
            w2 = pw.tile([128, 2 * NLOC], F16, tag="d")
            for t in range(2):
                ts = slice(t * NLOC, (t + 1) * NLOC)
                if route == "DVE":
                    nc.vector.scalar_tensor_tensor(
                        w2[:, ts], d2[:, ts], 1.0 / MBIG, m2[:, ts],
                        AO.mult, AO.mult)
                else:
                    nc.gpsimd.tensor_tensor(w2[:, ts], d2[:, ts], m2[:, ts],
                                            AO.mult)

        if prev is not None:
            emit_wv(*prev)
        prev = (w2, jta, jtb)
    emit_wv(*prev)

    # ---- finish: transpose via matmul with I65 (adds bv*Z), divide by Z ----
    nc.vector.tensor_copy(accT_sb[:], acc[:])
    for it in range(NLOC // 128):
        po = sbig[:, it % 2 * NLOC:it % 2 * NLOC + DQ + 1]
        nc.tensor.matmul(po, accT_sb[:, it * 128:(it + 1) * 128], i65_sb[:],
                         start=True, stop=True)
        rz = pfin.tile([128, 1], F32, tag="rz")
        nc.vector.reciprocal(rz[:], po[:, DQ:DQ + 1])
        o_t = pfin.tile([128, DQ], F32, tag="o")
        nc.vector.tensor_scalar_mul(o_t[:], po[:, 0:DQ], rz[:])
        nc.sync.dma_start(out[it * 128:(it + 1) * 128, :], o_t[:])


_CACHE = {}


def _program():
    if "nc" not in _CACHE:
        import contextlib
        nc = bacc.Bacc("TRN2", target_bir_lowering=False, debug=False,
                       num_devices=NCORES)
        with tile.TileContext(nc) as tc:
            with contextlib.ExitStack() as ctx:
                _emit(nc, tc, ctx)
        nc.compile()
        _CACHE["nc"] = nc
    return _CACHE["nc"]


def kernel(**inputs):
    x = np.asarray(inputs["x"], dtype=np.float32)
    ei = np.asarray(inputs["edge_index"])
    Wq = np.asarray(inputs["Wq"], dtype=np.float32)
    bq = np.asarray(inputs["bq"], dtype=np.float32)
    Wk = np.asarray(inputs["Wk"], dtype=np.float32)
    bk = np.asarray(inputs["bk"], dtype=np.float32)
    Wv = np.asarray(inputs["Wv"], dtype=np.float32)
    bv = np.asarray(inputs["bv"], dtype=np.float32)

    scale = 1.0 / np.sqrt(np.float32(DQ))
    f16 = ml_dtypes.float16 if not hasattr(np, "float16") else np.float16
    xT = np.ascontiguousarray(x.T).astype(np.float16)        # (256, 8192)
    wq_s = (Wq * scale).astype(np.float16)
    wqd = np.ascontiguousarray(np.concatenate([wq_s, wq_s], axis=1))
    wk16 = Wk.astype(np.float16)
    wkd = np.ascontiguousarray(np.concatenate([wk16, wk16], axis=1))
    wv16 = np.ascontiguousarray(Wv.astype(np.float16))
    bqd = np.ascontiguousarray(np.tile(bq * scale, 2).reshape(128, 1))
    bkd = np.ascontiguousarray(np.tile(bk, 2).reshape(128, 1))
    i65 = np.eye(DQ + 1, dtype=np.float32)
    i65[DQ, :DQ] = bv
    i65 = i65.astype(np.float16)
    i128 = np.eye(128, dtype=np.float32).astype(ml_dtypes.float8_e4m3)
    adj = np.zeros((N, N), dtype=np.bool_)
    adj[ei[0], ei[1]] = True

    in_maps = []
    for c in range(NCORES):
        rows = slice(c * NLOC, (c + 1) * NLOC)
        in_maps.append({
            "xt": xT,
            "xtq": np.ascontiguousarray(xT[:, rows]),
            "wqd": wqd, "wkd": wkd, "wv": wv16,
            "bqd": bqd, "bkd": bkd, "i65": i65, "i128": i128,
            "maskt": np.ascontiguousarray(
                adj[rows].T.astype(np.float32) * MBIG).astype(
                ml_dtypes.float8_e4m3),
        })

    global _last_in_maps
    _last_in_maps = in_maps
    nc = _program()
    res = run_bass_kernel_spmd(nc, in_maps, core_ids=list(range(NCORES)))
    out = np.concatenate([res.results[c]["out"] for c in range(NCORES)], axis=0)
    return out.astype(np.float32)


_last_in_maps = None
